# revision 8
# baseline (speedup 1.0000x reference)
"""
nn_CrossProdMean — Trainium2 Bass kernel (8 NeuronCores, data-parallel over batch).

Math:
    a = x @ Wa + ba                     # [b, n, hid]
    b = x @ Wb + bb                     # [b, n, hid]
    out = g * a * mean(b, -1, keepdims)

Key algebraic identity (exact):
    mean(b, -1) = x @ mean(Wb, axis=1) + mean(bb)
so the entire second matmul collapses to a matvec. Folding g into Wa/ba:
    out[t, h] = (x @ Wg + bg)[t, h] * m[t],   Wg = Wa * g,  bg = ba * g,
    m[t] = x[t] @ wbm + bbm,  wbm = Wb.mean(1), bbm = bb.mean()

Sharding: batch dim (8) -> one batch element per core. Weights replicated.

Device kernel (per core, x_c [4096, 1024]):
  - x is pre-transposed on host to xT [1024, 4096] so the contraction dim
    (1024) lands on SBUF partitions; xT 128x128 tiles are the stationary
    matmul operand, Wg columns the moving operand -> out tiles [tok, hid]
    are stored contiguously.
  - Matmuls run as float32r (1 cycle/row on the PE for N>=256, vs 4 for
    plain fp32).
  - epilogue on DVE: (+bias) then (*m per-partition scalar).
"""

import numpy as np

import concourse.bass as bass
import concourse.mybir as mybir
import concourse.tile as tile
from concourse import bacc
from concourse.bass_utils import run_bass_kernel_spmd

F32 = mybir.dt.float32
F32R = mybir.dt.float32r

N_CORES = 8
TOK = 4096          # tokens per core (batch element)
DIM = 1024
HID = 1024
P = 128
K_TILES = DIM // P          # 8
N_FREE = 512                # moving free dim per matmul
N_TILES = HID // N_FREE     # 2
SLAB = 512                  # tokens per x DMA slab
N_SLABS = TOK // SLAB       # 8
J_PER_SLAB = SLAB // P      # 4
MPAD = 8                    # moving-dim padding for the m matvec

# matmul input dtype for the big y matmuls: F32R (fast) or F32 (4x slower, exact)
Y_DT = F32R


def _build_module(reps=1):
    nc = bacc.Bacc("TRN2", target_bir_lowering=False, debug=False)

    xt = nc.dram_tensor("xt", [DIM, TOK], Y_DT, kind="ExternalInput")
    wg = nc.dram_tensor("wg", [DIM, HID], Y_DT, kind="ExternalInput")
    bg = nc.dram_tensor("bg", [P, HID], F32, kind="ExternalInput")
    wbm = nc.dram_tensor("wbm", [DIM, MPAD], Y_DT, kind="ExternalInput")
    bbm = nc.dram_tensor("bbm", [P, 1], F32, kind="ExternalInput")
    out = nc.dram_tensor("out", [TOK, HID], F32, kind="ExternalOutput")

    xt_r = xt[:].rearrange("(k p) t -> p k t", p=P)
    wg_r = wg[:].rearrange("(k p) h -> p k h", p=P)
    wbm_r = wbm[:].rearrange("(k p) o -> p k o", p=P)
    out_r = out[:].rearrange("(s j p) h -> s p j h", p=P, j=J_PER_SLAB)

    with tile.TileContext(nc) as tc:
        with (
            tc.tile_pool(name="const", bufs=1) as const_pool,
            tc.tile_pool(name="xtp", bufs=3) as x_pool,
            tc.tile_pool(name="outp", bufs=2) as out_pool,
            tc.tile_pool(name="tmpp", bufs=4) as tmp_pool,
            tc.tile_pool(name="mp", bufs=8) as m_pool,
            tc.tile_pool(name="psy", bufs=6, space="PSUM") as psum_y_pool,
            tc.tile_pool(name="psm", bufs=2, space="PSUM") as psum_m_pool,
        ):

            def body():
                bbm_sb = const_pool.tile([P, 1], F32, tag="bbm")
                nc.sync.dma_start(bbm_sb[:], bbm[:])
                wbm_sb = const_pool.tile([P, K_TILES, MPAD], Y_DT, tag="wbm")
                nc.sync.dma_start(wbm_sb[:], wbm_r)
                bg_sb = const_pool.tile([P, HID], F32, tag="bg")
                nc.sync.dma_start(bg_sb[:], bg[:])
                wg_sb = const_pool.tile([P, K_TILES, HID], Y_DT, tag="wg")
                # split so the first matmul group's weights land sooner
                nc.sync.dma_start(
                    wg_sb[:, :, 0:N_FREE], wg_r[:, :, 0:N_FREE]
                )
                nc.sync.dma_start(
                    wg_sb[:, :, N_FREE:HID], wg_r[:, :, N_FREE:HID]
                )

                for s in range(N_SLABS):
                    xt_sb = x_pool.tile([P, K_TILES, SLAB], Y_DT, tag="xt")
                    nc.sync.dma_start(
                        xt_sb[:], xt_r[:, :, s * SLAB:(s + 1) * SLAB]
                    )
                    out_sb = out_pool.tile([P, J_PER_SLAB, HID], F32,
                                           tag="os")
                    for j in range(J_PER_SLAB):
                        lhsT = [
                            xt_sb[:, k, j * P:(j + 1) * P]
                            for k in range(K_TILES)
                        ]
                        psum_y0 = psum_y_pool.tile([P, N_FREE], F32, tag="py")
                        psum_y1 = psum_y_pool.tile([P, N_FREE], F32, tag="py")
                        psum_m = psum_m_pool.tile([P, MPAD], F32, tag="pm")
                        for k in range(K_TILES):
                            st = k == 0
                            sp = k == K_TILES - 1
                            nc.tensor.matmul(
                                psum_y0[:],
                                lhsT[k],
                                wg_sb[:, k, 0:N_FREE],
                                start=st, stop=sp,
                            )
                            nc.tensor.matmul(
                                psum_y1[:],
                                lhsT[k],
                                wg_sb[:, k, N_FREE:HID],
                                start=st, stop=sp,
                            )
                            nc.tensor.matmul(
                                psum_m[:],
                                lhsT[k],
                                wbm_sb[:, k, :],
                                start=st, stop=sp,
                            )
                        m_sb = m_pool.tile([P, 1], F32, tag="m")
                        nc.vector.tensor_add(m_sb[:], psum_m[:, 0:1],
                                             bbm_sb[:])

                        for n, psum_y in ((0, psum_y0), (1, psum_y1)):
                            hs0 = n * N_FREE
                            tmp = tmp_pool.tile([P, N_FREE], F32, tag="t")
                            nc.vector.tensor_add(
                                tmp[:], psum_y[:], bg_sb[:, hs0:hs0 + N_FREE]
                            )
                            nc.vector.tensor_scalar_mul(
                                out_sb[:, j, hs0:hs0 + N_FREE], tmp[:],
                                m_sb[:]
                            )
                    nc.sync.dma_start(out_r[s], out_sb[:])

            if reps == 1:
                body()
            else:
                with tc.For_i(0, reps, 1):
                    body()

    nc.compile()
    return nc


_NC = None


def _get_module():
    global _NC
    if _NC is None:
        _NC = _build_module()
    return _NC


def _round_fp32r(a):
    """Round fp32 array to fp32r (e8m11: RNE to 11 mantissa bits, low 12
    bits zeroed) — the PE's fast single-pass fp32 matmul input format."""
    u = np.ascontiguousarray(a, dtype=np.float32).view(np.uint32)
    lsb = (u >> 12) & 1
    r = (u + 0x7FF + lsb) & np.uint32(0xFFFFF000)
    return r.view(np.float32)


def _prep_inputs(x, Wa, ba, Wb, bb, g):
    x = np.asarray(x, dtype=np.float32)
    Wa = np.asarray(Wa, dtype=np.float32)
    ba = np.asarray(ba, dtype=np.float32)
    Wb = np.asarray(Wb, dtype=np.float32)
    bb = np.asarray(bb, dtype=np.float32)
    g = np.asarray(g, dtype=np.float32)

    round_in = _round_fp32r if Y_DT == F32R else (
        lambda a: np.ascontiguousarray(a, dtype=np.float32))

    wg = round_in(Wa * g[None, :])
    bg_row = ba * g
    bg = np.ascontiguousarray(np.broadcast_to(bg_row[None, :], (P, HID)))
    wbm_vec = Wb.mean(axis=1, dtype=np.float64).astype(np.float32)
    wbm_pad = np.zeros((DIM, MPAD), dtype=np.float32)
    wbm_pad[:, 0] = wbm_vec
    wbm = round_in(wbm_pad)
    bbm_val = np.float32(bb.mean(dtype=np.float64))
    bbm = np.full((P, 1), bbm_val, dtype=np.float32)

    in_maps = []
    for c in range(N_CORES):
        xt_c = round_in(np.ascontiguousarray(x[c].T))
        in_maps.append({
            "xt": xt_c, "wg": wg, "bg": bg, "wbm": wbm, "bbm": bbm,
        })
    return in_maps


def kernel(x, Wa, ba, Wb, bb, g):
    nc = _get_module()
    in_maps = _prep_inputs(x, Wa, ba, Wb, bb, g)
    res = run_bass_kernel_spmd(nc, in_maps, list(range(N_CORES)))
    out = np.stack([res.results[c]["out"] for c in range(N_CORES)], axis=0)
    return out
